# revision 5
# baseline (speedup 1.0000x reference)
r"""Trainium2 Bass kernel for the triangular-DP "MAA layer" problem.

Reference computes, per frame t (T=1024, D=256, L=T+1 counts):
    q_t = (1-p_t) q_{t-1} + p_t shift(q_{t-1})          (Poisson-binomial DP)
    m_t = p_t a m_sh + (1-p_t) m + p_t b q_sh x_t       ([L, D] state)
    out = sum_i m_T[i, :]                               ([D])

Algebraic restructuring: the whole scan collapses to

    out[d] = sum_t c_t x[t, d],
    c_t    = p_t * I_t,   I_t = int_0^1 prod_{s != t} ((1-p_s) + p_s u) du.

The integrand is a boundary-layer spike at u=1 of width ~1/S, S = sum_s p_s.
Gauss-Legendre on the rescaled interval [1 - 30/S, 1] (host-computed from p)
converges at K=16 nodes to ~1e-12 (tail cut error e^-30).  With
f[t,k] = 1 + p_t (u_k - 1):

    slog_k = sum_t ln f[t,k]
    c_t    = p_t * sum_k exp(slog_k + ln w_k - ln f[t,k])
    out    = c^T @ x

Device mapping (t on partitions, 8 chunks of 128; k on free dim, K=16),
replicated on all 8 cores (collective latency floor exceeds compute):
  - p lands directly as pcol [128,8] via a single-packet DMA; the same
    tensor carries the quadrature constants in row 0's spare columns
  - row->partition broadcasts on GpSimd (ucode warmed by a dummy), not PE
  - single big Ln and single big Exp on ScalarE; the Exp table load hides
    behind the DVE halving-adds + PE slog matmul
  - final contraction: 8 accumulating bf16 PE matmuls (x shipped as bf16)
  - a couple of junk matmuls lift the PE out of its lowest p-state early
"""

import numpy as np

T, D, NCH, P, K = 1024, 256, 8, 128, 16
N_CORES = 8

_CACHE = {}


def _build_program():
    import concourse.bass as bass
    import concourse.bacc as bacc
    import concourse.mybir as mybir
    import concourse.tile as tile

    f32 = mybir.dt.float32
    bf16 = mybir.dt.bfloat16
    A = mybir.AluOpType
    ACT = mybir.ActivationFunctionType

    nc = bacc.Bacc("TRN2", target_bir_lowering=False, debug=False,
                   num_devices=N_CORES)

    AUXW = NCH + 2 * K  # [pcol(8) | um1(16) row0 | lnw(16) row0]
    pcaux_d = nc.dram_tensor("pcaux", [P, AUXW], f32, kind="ExternalInput")
    xa_d = nc.dram_tensor("xa", [P, NCH * D], bf16, kind="ExternalInput")
    out_d = nc.dram_tensor("out", [1, D], f32, kind="ExternalOutput")

    with tile.TileContext(nc) as tc:
        with (
            tc.tile_pool(name="sb", bufs=1) as sb,
            tc.tile_pool(name="ps", bufs=1, space=bass.MemorySpace.PSUM) as ps,
        ):
            pcaux = sb.tile([P, AUXW], f32, tag="pcaux")
            xa = sb.tile([P, NCH * D], bf16, tag="xa")
            nc.sync.dma_start(pcaux[:], pcaux_d[:], single_packet=True)
            nc.sync.dma_start(xa[:], xa_d[:])
            pcol = pcaux[:, 0:NCH]
            um1row = pcaux[0:1, NCH:NCH + K]
            lnwrow = pcaux[0:1, NCH + K:NCH + 2 * K]

            onescol = sb.tile([P, 1], f32, tag="onescol")
            nc.gpsimd.memset(onescol[:], 1.0)

            # GpSimd ucode warmup for partition_broadcast (one-time setup)
            dumb = sb.tile([NCH, K], f32, tag="dumb")
            nc.gpsimd.memset(dumb[0:1, :], 0.0)
            nc.gpsimd.partition_broadcast(dumb[:], dumb[0:1, :], channels=NCH)

            # PE warmup: junk matmuls lift PE out of the lowest p-state
            jmv = sb.tile([P, 512], bf16, tag="jmv")
            nc.vector.memset(jmv[:], 0.0)
            jps = ps.tile([1, 512], f32, tag="jps")
            nc.tensor.matmul(jps[:], jmv[:, 0:1], jmv[:],
                             start=True, stop=True)
            nc.tensor.matmul(jps[:], jmv[:, 0:1], jmv[:],
                             start=True, stop=True)

            # um1bc[t, k] = u_k - 1 broadcast across partitions (GpSimd)
            um1bc = sb.tile([P, K], f32, tag="um1bc")
            nc.gpsimd.partition_broadcast(um1bc[:], um1row, channels=P)

            # fm1[t,(c,k)] = pcol[t,c] * um1[k]
            fm1 = sb.tile([P, NCH * K], f32, tag="fm1")
            um1_rep = um1bc.unsqueeze(1).broadcast_to([P, NCH, K])
            p_rep = pcol.unsqueeze(2).broadcast_to([P, NCH, K])
            nc.vector.tensor_tensor(fm1.rearrange("p (c k) -> p c k", c=NCH),
                                    um1_rep, p_rep, op=A.mult)

            # lf = Ln(fm1 + 1)
            lfbig = sb.tile([P, NCH * K], f32, tag="lfbig")
            nc.scalar.activation(lfbig[:], fm1[:], ACT.Ln, bias=1.0)

            # lfsum[t, k] = sum_c lf[t,(c,k)] via 3 contiguous halving adds
            h1 = sb.tile([P, 4 * K], f32, tag="h1")
            nc.vector.tensor_tensor(h1[:], lfbig[:, 0:4 * K],
                                    lfbig[:, 4 * K:8 * K], op=A.add)
            h2 = sb.tile([P, 2 * K], f32, tag="h2")
            nc.vector.tensor_tensor(h2[:], h1[:, 0:2 * K], h1[:, 2 * K:4 * K],
                                    op=A.add)
            lfsum = sb.tile([P, K], f32, tag="lfsum")
            nc.vector.tensor_tensor(lfsum[:], h2[:, 0:K], h2[:, K:2 * K],
                                    op=A.add)

            # slog_k = sum_t lfsum[t, k]  (PE partition reduce)
            slog_ps = ps.tile([1, K], f32, tag="slog_ps")
            nc.tensor.matmul(slog_ps[:], onescol[:], lfsum[:],
                             start=True, stop=True)

            # slnw = slog + ln w ; broadcast to partitions (GpSimd)
            slnw = sb.tile([1, K], f32, tag="slnw")
            nc.vector.tensor_tensor(slnw[:], slog_ps[:], lnwrow, op=A.add)
            argbc = sb.tile([P, K], f32, tag="argbc")
            nc.gpsimd.partition_broadcast(argbc[:], slnw[:], channels=P)

            # arg = (slog + lnw) - lf ;  e = exp(arg)
            arg = sb.tile([P, NCH * K], f32, tag="arg")
            argbc_rep = argbc.unsqueeze(1).broadcast_to([P, NCH, K])
            nc.vector.tensor_tensor(arg.rearrange("p (c k) -> p c k", c=NCH),
                                    argbc_rep,
                                    lfbig.rearrange("p (c k) -> p c k", c=NCH),
                                    op=A.subtract)
            e = sb.tile([P, NCH * K], f32, tag="e")
            nc.scalar.activation(e[:], arg[:], ACT.Exp)

            # cfin[t, c] = pcol[t,c] * sum_k e[t,(c,k)]  (bf16 out, fused)
            cfin8 = sb.tile([P, NCH], f32, tag="cfin8")
            nc.vector.tensor_reduce(
                cfin8[:], e.rearrange("p (c k) -> p c k", c=NCH),
                axis=mybir.AxisListType.X, op=A.add)
            cfinb = sb.tile([P, NCH], bf16, tag="cfinb")
            nc.vector.tensor_tensor(cfinb[:], cfin8[:], pcol[:], op=A.mult)

            # out[d] = sum_c sum_t cfin[t,c] x[t,(c,d)] : 8 accumulating MMs
            out_ps = ps.tile([1, D], f32, tag="out_ps")
            for c in range(NCH):
                nc.tensor.matmul(out_ps[:], cfinb[:, c:c + 1],
                                 xa[:, c * D:(c + 1) * D],
                                 start=(c == 0), stop=(c == NCH - 1))
            # psum -> sbuf in two parallel halves (DVE + ScalarE)
            out_sb = sb.tile([1, D], f32, tag="outsb")
            nc.vector.tensor_copy(out_sb[:, 0:D // 2], out_ps[:, 0:D // 2])
            nc.scalar.activation(out_sb[:, D // 2:D], out_ps[:, D // 2:D],
                                 ACT.Copy)
            nc.sync.dma_start(out_d[:], out_sb[:], single_packet=True)

    nc.compile()
    return nc


def _make_in_map(p, x):
    import ml_dtypes

    p = np.ascontiguousarray(np.asarray(p, dtype=np.float32)).reshape(T)
    x = np.ascontiguousarray(np.asarray(x, dtype=np.float32)).reshape(T, D)
    S = float(np.sum(np.asarray(p, np.float64)))
    delta = min(1.0, 30.0 / max(S, 1.0))
    nodes, weights = np.polynomial.legendre.leggauss(K)
    u = 1.0 - delta + delta * (nodes + 1.0) * 0.5
    w = weights * delta * 0.5
    pcaux = np.zeros((P, NCH + 2 * K), np.float32)
    pcaux[:, 0:NCH] = p.reshape(NCH, P).T
    pcaux[0, NCH:NCH + K] = (u - 1.0).astype(np.float32)
    pcaux[0, NCH + K:NCH + 2 * K] = np.log(w).astype(np.float32)
    xa = np.ascontiguousarray(
        x.reshape(NCH, P, D).transpose(1, 0, 2).reshape(P, NCH * D)
    ).astype(ml_dtypes.bfloat16)
    return {"pcaux": pcaux, "xa": xa}


def _run(p, x, trace=False, tmpdir=None):
    from concourse.bass_utils import run_bass_kernel_spmd

    if "nc" not in _CACHE:
        _CACHE["nc"] = _build_program()
    nc = _CACHE["nc"]
    in_map = _make_in_map(p, x)
    in_maps = [in_map for _ in range(N_CORES)]
    res = run_bass_kernel_spmd(nc, in_maps, list(range(N_CORES)),
                               trace=trace, tmpdir=tmpdir)
    out = np.asarray(res.results[0]["out"], dtype=np.float32).reshape(D)
    return out, res


def kernel(p, x):
    out, _ = _run(p, x, trace=False)
    return out


# revision 6
# speedup vs baseline: 1.1998x; 1.1998x over previous
r"""Trainium2 Bass kernel for the triangular-DP "MAA layer" problem.

Reference computes, per frame t (T=1024, D=256, L=T+1 counts):
    q_t = (1-p_t) q_{t-1} + p_t shift(q_{t-1})          (Poisson-binomial DP)
    m_t = p_t a m_sh + (1-p_t) m + p_t b q_sh x_t       ([L, D] state)
    out = sum_i m_T[i, :]                               ([D])

Algebraic restructuring: the whole scan collapses to

    out[d] = sum_t c_t x[t, d],
    c_t    = p_t * I_t,   I_t = int_0^1 prod_{s != t} ((1-p_s) + p_s u) du.

The integrand is a boundary-layer spike at u=1 of width ~1/S, S = sum_s p_s.
Gauss-Legendre on the rescaled interval [1 - 30/S, 1] (host-computed from p)
converges at K=16 nodes to ~1e-12 (tail cut error e^-30).  With
f[t,k] = 1 + p_t (u_k - 1):

    slog_k = sum_t ln f[t,k]
    c_t    = p_t * sum_k exp(slog_k + ln w_k - ln f[t,k])
    out    = c^T @ x

Device mapping (t on partitions, 8 chunks of 128; k on free dim, K=16),
replicated on all 8 cores (collective latency floor exceeds compute):
  - one [128, 8+16+16] aux input carries pcol + host-pre-broadcast um1/lnw
    (no device-side row broadcasts at all), issued on the GpSimd engine's
    DMA queue so its packets flow in parallel with the x stream
  - the slog partition-reduce uses a [128,128] all-ones stationary so its
    PSUM output lands already broadcast across partitions
  - single big Ln and single big Exp on ScalarE; the Exp table load hides
    behind the DVE halving-adds + the slog matmul
  - final contraction: 8 accumulating bf16 PE matmuls (x shipped as bf16)
  - two junk matmuls lift the PE out of its lowest p-state early
"""

import numpy as np

T, D, NCH, P, K = 1024, 256, 8, 128, 16
N_CORES = 8

_CACHE = {}


def _build_program():
    import concourse.bass as bass
    import concourse.bacc as bacc
    import concourse.mybir as mybir
    import concourse.tile as tile

    f32 = mybir.dt.float32
    bf16 = mybir.dt.bfloat16
    A = mybir.AluOpType
    ACT = mybir.ActivationFunctionType

    nc = bacc.Bacc("TRN2", target_bir_lowering=False, debug=False,
                   num_devices=N_CORES)

    AUXW = NCH + 2 * K  # [pcol(8) | um1bc(16) | lnwbc(16)], host-broadcast
    paux_d = nc.dram_tensor("paux", [P, AUXW], f32, kind="ExternalInput")
    xa_d = nc.dram_tensor("xa", [P, NCH * D], bf16, kind="ExternalInput")
    out_d = nc.dram_tensor("out", [1, D], f32, kind="ExternalOutput")

    with tile.TileContext(nc) as tc:
        with (
            tc.tile_pool(name="sb", bufs=1) as sb,
            tc.tile_pool(name="ps", bufs=1, space=bass.MemorySpace.PSUM) as ps,
        ):
            paux = sb.tile([P, AUXW], f32, tag="paux")
            xa = sb.tile([P, NCH * D], bf16, tag="xa")
            # parallel descriptor streams: aux on GpSimd's queue, x on Sync's
            nc.gpsimd.dma_start(paux[:], paux_d[:])
            nc.sync.dma_start(xa[:], xa_d[:])
            pcol = paux[:, 0:NCH]
            um1bc = paux[:, NCH:NCH + K]
            lnwbc = paux[:, NCH + K:NCH + 2 * K]

            onesbig = sb.tile([P, P], f32, tag="onesbig")
            nc.gpsimd.memset(onesbig[:], 1.0)

            # PE warmup: junk matmuls lift PE out of the lowest p-state
            jmv = sb.tile([P, 512], bf16, tag="jmv")
            nc.gpsimd.memset(jmv[:], 0.0)
            jps = ps.tile([1, 512], f32, tag="jps")
            nc.tensor.matmul(jps[:], jmv[:, 0:1], jmv[:],
                             start=True, stop=True)
            nc.tensor.matmul(jps[:], jmv[:, 0:1], jmv[:],
                             start=True, stop=True)

            # fm1[t,(c,k)] = pcol[t,c] * um1[k]
            fm1 = sb.tile([P, NCH * K], f32, tag="fm1")
            um1_rep = um1bc.unsqueeze(1).broadcast_to([P, NCH, K])
            p_rep = pcol.unsqueeze(2).broadcast_to([P, NCH, K])
            nc.vector.tensor_tensor(fm1.rearrange("p (c k) -> p c k", c=NCH),
                                    um1_rep, p_rep, op=A.mult)

            # lf = Ln(fm1 + 1)
            lfbig = sb.tile([P, NCH * K], f32, tag="lfbig")
            nc.scalar.activation(lfbig[:], fm1[:], ACT.Ln, bias=1.0)

            # lfsum[t, k] = sum_c lf[t,(c,k)] via 3 contiguous halving adds
            h1 = sb.tile([P, 4 * K], f32, tag="h1")
            nc.vector.tensor_tensor(h1[:], lfbig[:, 0:4 * K],
                                    lfbig[:, 4 * K:8 * K], op=A.add)
            h2 = sb.tile([P, 2 * K], f32, tag="h2")
            nc.vector.tensor_tensor(h2[:], h1[:, 0:2 * K], h1[:, 2 * K:4 * K],
                                    op=A.add)
            lfsum = sb.tile([P, K], f32, tag="lfsum")
            nc.vector.tensor_tensor(lfsum[:], h2[:, 0:K], h2[:, K:2 * K],
                                    op=A.add)

            # slogbc[i, k] = sum_t lfsum[t, k] for every i: all-ones
            # stationary makes the PE reduce land pre-broadcast in PSUM
            slogbc_ps = ps.tile([P, K], f32, tag="slogbc_ps")
            nc.tensor.matmul(slogbc_ps[:], onesbig[:], lfsum[:],
                             start=True, stop=True)

            # w2 = slog + ln w (broadcast), then arg = w2 - lf ; e = exp(arg)
            w2 = sb.tile([P, K], f32, tag="w2")
            nc.vector.tensor_tensor(w2[:], slogbc_ps[:], lnwbc, op=A.add)
            arg = sb.tile([P, NCH * K], f32, tag="arg")
            w2_rep = w2.unsqueeze(1).broadcast_to([P, NCH, K])
            nc.vector.tensor_tensor(arg.rearrange("p (c k) -> p c k", c=NCH),
                                    w2_rep,
                                    lfbig.rearrange("p (c k) -> p c k", c=NCH),
                                    op=A.subtract)
            e = sb.tile([P, NCH * K], f32, tag="e")
            nc.scalar.activation(e[:], arg[:], ACT.Exp)

            # cfin[t, c] = pcol[t,c] * sum_k e[t,(c,k)]  (bf16 out, fused)
            cfin8 = sb.tile([P, NCH], f32, tag="cfin8")
            nc.vector.tensor_reduce(
                cfin8[:], e.rearrange("p (c k) -> p c k", c=NCH),
                axis=mybir.AxisListType.X, op=A.add)
            cfinb = sb.tile([P, NCH], bf16, tag="cfinb")
            nc.vector.tensor_tensor(cfinb[:], cfin8[:], pcol[:], op=A.mult)

            # out[d] = sum_c sum_t cfin[t,c] x[t,(c,d)] : 8 accumulating MMs
            out_ps = ps.tile([1, D], f32, tag="out_ps")
            for c in range(NCH):
                nc.tensor.matmul(out_ps[:], cfinb[:, c:c + 1],
                                 xa[:, c * D:(c + 1) * D],
                                 start=(c == 0), stop=(c == NCH - 1))
            # psum -> sbuf in two parallel halves (DVE + ScalarE)
            out_sb = sb.tile([1, D], f32, tag="outsb")
            nc.vector.tensor_copy(out_sb[:, 0:D // 2], out_ps[:, 0:D // 2])
            nc.scalar.activation(out_sb[:, D // 2:D], out_ps[:, D // 2:D],
                                 ACT.Copy)
            nc.sync.dma_start(out_d[:], out_sb[:])

    nc.compile()
    return nc


def _make_in_map(p, x):
    import ml_dtypes

    p = np.ascontiguousarray(np.asarray(p, dtype=np.float32)).reshape(T)
    x = np.ascontiguousarray(np.asarray(x, dtype=np.float32)).reshape(T, D)
    S = float(np.sum(np.asarray(p, np.float64)))
    delta = min(1.0, 30.0 / max(S, 1.0))
    nodes, weights = np.polynomial.legendre.leggauss(K)
    u = 1.0 - delta + delta * (nodes + 1.0) * 0.5
    w = weights * delta * 0.5
    paux = np.empty((P, NCH + 2 * K), np.float32)
    paux[:, 0:NCH] = p.reshape(NCH, P).T
    paux[:, NCH:NCH + K] = (u - 1.0).astype(np.float32)[None, :]
    paux[:, NCH + K:NCH + 2 * K] = np.log(w).astype(np.float32)[None, :]
    xa = np.ascontiguousarray(
        x.reshape(NCH, P, D).transpose(1, 0, 2).reshape(P, NCH * D)
    ).astype(ml_dtypes.bfloat16)
    return {"paux": paux, "xa": xa}


def _run(p, x, trace=False, tmpdir=None):
    from concourse.bass_utils import run_bass_kernel_spmd

    if "nc" not in _CACHE:
        _CACHE["nc"] = _build_program()
    nc = _CACHE["nc"]
    in_map = _make_in_map(p, x)
    in_maps = [in_map for _ in range(N_CORES)]
    res = run_bass_kernel_spmd(nc, in_maps, list(range(N_CORES)),
                               trace=trace, tmpdir=tmpdir)
    out = np.asarray(res.results[0]["out"], dtype=np.float32).reshape(D)
    return out, res


def kernel(p, x):
    out, _ = _run(p, x, trace=False)
    return out


# revision 11
# speedup vs baseline: 1.3221x; 1.1019x over previous
r"""Trainium2 Bass kernel for the triangular-DP "MAA layer" problem.

Reference computes, per frame t (T=1024, D=256, L=T+1 counts):
    q_t = (1-p_t) q_{t-1} + p_t shift(q_{t-1})          (Poisson-binomial DP)
    m_t = p_t a m_sh + (1-p_t) m + p_t b q_sh x_t       ([L, D] state)
    out = sum_i m_T[i, :]                               ([D])

Algebraic restructuring: the whole scan collapses to

    out[d] = sum_t c_t x[t, d],
    c_t    = p_t * I_t,   I_t = int_0^1 prod_{s != t} ((1-p_s) + p_s u) du.

The integrand is a boundary-layer spike at u=1 of width ~1/S, S = sum_s p_s.
Gauss-Legendre on the rescaled interval [1 - 30/S, 1] (host-computed from p)
converges at K=16 nodes to ~1e-12 (tail cut error e^-30).  With
f[t,k] = 1 + p_t (u_k - 1):

    slog_k = sum_t ln f[t,k]
    c_t    = p_t * sum_k exp(slog_k + ln w_k - ln f[t,k])
    out    = c^T @ x

Device mapping (t on partitions, 8 chunks of 128; k on free dim, K=16),
replicated on all 8 cores (collective latency floor exceeds compute):
  - one [128, 8+16+16] aux input carries pcol + host-pre-broadcast um1/lnw
    (no device-side row broadcasts at all), issued on the GpSimd engine's
    DMA queue so its packets flow in parallel with the x stream
  - the slog partition-reduce uses a [128,128] all-ones stationary so its
    PSUM output lands already broadcast across partitions
  - single big Ln and single big Exp on ScalarE; the Exp table load hides
    behind the DVE halving-adds + the slog matmul
  - final contraction: 8 accumulating bf16 PE matmuls (x shipped as bf16)
  - two junk matmuls lift the PE out of its lowest p-state early
"""

import numpy as np

T, D, NCH, P, K = 1024, 256, 8, 128, 16
N_CORES = 8

_CACHE = {}


def _build_program():
    import concourse.bass as bass
    import concourse.bacc as bacc
    import concourse.mybir as mybir
    import concourse.tile as tile

    f32 = mybir.dt.float32
    bf16 = mybir.dt.bfloat16
    A = mybir.AluOpType
    ACT = mybir.ActivationFunctionType

    nc = bacc.Bacc("TRN2", target_bir_lowering=False, debug=False,
                   num_devices=N_CORES)

    AUXW = NCH + 2 * K  # [pcol(8) | um1bc(16) | lnwbc(16)], host-broadcast
    paux_d = nc.dram_tensor("paux", [P, AUXW], f32, kind="ExternalInput")
    xa_d = nc.dram_tensor("xa", [P, NCH * D], bf16, kind="ExternalInput")
    out_d = nc.dram_tensor("out", [1, D], f32, kind="ExternalOutput")

    with tile.TileContext(nc) as tc:
        with (
            tc.tile_pool(name="sb", bufs=1) as sb,
            tc.tile_pool(name="ps", bufs=1, space=bass.MemorySpace.PSUM) as ps,
        ):
            paux = sb.tile([P, AUXW], f32, tag="paux")
            xa = sb.tile([P, NCH * D], bf16, tag="xa")
            # paux split across two engines' DMA queues so the small-packet
            # streams flow in parallel; xa second on Sync's queue
            nc.sync.dma_start(paux[0:64, :], paux_d[0:64, :])
            nc.scalar.dma_start(paux[64:P, :], paux_d[64:P, :])
            nc.sync.dma_start(xa[:], xa_d[:])
            pcol = paux[:, 0:NCH]
            um1bc = paux[:, NCH:NCH + K]
            lnwbc = paux[:, NCH + K:NCH + 2 * K]

            onesbig = sb.tile([P, P], f32, tag="onesbig")
            nc.gpsimd.memset(onesbig[:], 1.0)

            # PE warmup: junk matmuls lift PE out of the lowest p-state
            jmv = sb.tile([P, 512], bf16, tag="jmv")
            nc.gpsimd.memset(jmv[:], 0.0)
            jps = ps.tile([1, 512], f32, tag="jps")
            nc.tensor.matmul(jps[:], jmv[:, 0:1], jmv[:],
                             start=True, stop=True)
            nc.tensor.matmul(jps[:], jmv[:, 0:1], jmv[:],
                             start=True, stop=True)

            # PSUM preload for the slog matmul: lnw (already host-broadcast)
            # goes in first, the partition-reduce then accumulates on top
            slogbc_ps = ps.tile([P, K], f32, tag="slogbc_ps")
            nc.vector.tensor_copy(slogbc_ps[:], lnwbc)

            # fm1[t,(c,k)] = pcol[t,c] * um1[k]
            fm1 = sb.tile([P, NCH * K], f32, tag="fm1")
            um1_rep = um1bc.unsqueeze(1).broadcast_to([P, NCH, K])
            p_rep = pcol.unsqueeze(2).broadcast_to([P, NCH, K])
            nc.vector.tensor_tensor(fm1.rearrange("p (c k) -> p c k", c=NCH),
                                    um1_rep, p_rep, op=A.mult)

            # lf = Ln(fm1 + 1)
            lfbig = sb.tile([P, NCH * K], f32, tag="lfbig")
            nc.scalar.activation(lfbig[:], fm1[:], ACT.Ln, bias=1.0)

            # lfsum[t, k] = sum_c lf[t,(c,k)] via 3 contiguous halving adds
            h1 = sb.tile([P, 4 * K], f32, tag="h1")
            nc.vector.tensor_tensor(h1[:], lfbig[:, 0:4 * K],
                                    lfbig[:, 4 * K:8 * K], op=A.add)
            h2 = sb.tile([P, 2 * K], f32, tag="h2")
            nc.vector.tensor_tensor(h2[:], h1[:, 0:2 * K], h1[:, 2 * K:4 * K],
                                    op=A.add)
            lfsum = sb.tile([P, K], f32, tag="lfsum")
            nc.vector.tensor_tensor(lfsum[:], h2[:, 0:K], h2[:, K:2 * K],
                                    op=A.add)

            # slogbc[i, k] = lnw[k] + sum_t lfsum[t, k] for every i: the
            # all-ones stationary makes the PE reduce land pre-broadcast,
            # accumulating on top of the preloaded lnw
            nc.tensor.matmul(slogbc_ps[:], onesbig[:], lfsum[:],
                             start=False, stop=True, skip_group_check=True)

            # arg = (slog + lnw) - lf ; e = exp(arg)
            arg = sb.tile([P, NCH * K], f32, tag="arg")
            w2_rep = slogbc_ps.unsqueeze(1).broadcast_to([P, NCH, K])
            nc.vector.tensor_tensor(arg.rearrange("p (c k) -> p c k", c=NCH),
                                    w2_rep,
                                    lfbig.rearrange("p (c k) -> p c k", c=NCH),
                                    op=A.subtract)
            e = sb.tile([P, NCH * K], f32, tag="e")
            nc.scalar.activation(e[:], arg[:], ACT.Exp)

            # cfin[t, c] = pcol[t,c] * sum_k e[t,(c,k)]  (bf16 out, fused)
            cfin8 = sb.tile([P, NCH], f32, tag="cfin8")
            nc.vector.tensor_reduce(
                cfin8[:], e.rearrange("p (c k) -> p c k", c=NCH),
                axis=mybir.AxisListType.X, op=A.add)
            cfinb = sb.tile([P, NCH], bf16, tag="cfinb")
            nc.vector.tensor_tensor(cfinb[:], cfin8[:], pcol[:], op=A.mult)

            # out[d] = sum_c sum_t cfin[t,c] x[t,(c,d)] : 8 accumulating MMs
            out_ps = ps.tile([1, D], f32, tag="out_ps")
            for c in range(NCH):
                nc.tensor.matmul(out_ps[:], cfinb[:, c:c + 1],
                                 xa[:, c * D:(c + 1) * D],
                                 start=(c == 0), stop=(c == NCH - 1))
            # psum -> sbuf in two parallel halves (DVE + ScalarE)
            out_sb = sb.tile([1, D], f32, tag="outsb")
            nc.vector.tensor_copy(out_sb[:, 0:D // 2], out_ps[:, 0:D // 2])
            nc.scalar.activation(out_sb[:, D // 2:D], out_ps[:, D // 2:D],
                                 ACT.Copy)
            nc.sync.dma_start(out_d[:], out_sb[:])

    nc.compile()
    return nc


def _make_in_map(p, x):
    import ml_dtypes

    p = np.ascontiguousarray(np.asarray(p, dtype=np.float32)).reshape(T)
    x = np.ascontiguousarray(np.asarray(x, dtype=np.float32)).reshape(T, D)
    S = float(np.sum(np.asarray(p, np.float64)))
    delta = min(1.0, 30.0 / max(S, 1.0))
    nodes, weights = np.polynomial.legendre.leggauss(K)
    u = 1.0 - delta + delta * (nodes + 1.0) * 0.5
    w = weights * delta * 0.5
    paux = np.empty((P, NCH + 2 * K), np.float32)
    paux[:, 0:NCH] = p.reshape(NCH, P).T
    paux[:, NCH:NCH + K] = (u - 1.0).astype(np.float32)[None, :]
    paux[:, NCH + K:NCH + 2 * K] = np.log(w).astype(np.float32)[None, :]
    xa = np.ascontiguousarray(
        x.reshape(NCH, P, D).transpose(1, 0, 2).reshape(P, NCH * D)
    ).astype(ml_dtypes.bfloat16)
    return {"paux": paux, "xa": xa}


def _run(p, x, trace=False, tmpdir=None):
    from concourse.bass_utils import run_bass_kernel_spmd

    if "nc" not in _CACHE:
        _CACHE["nc"] = _build_program()
    nc = _CACHE["nc"]
    in_map = _make_in_map(p, x)
    in_maps = [in_map for _ in range(N_CORES)]
    res = run_bass_kernel_spmd(nc, in_maps, list(range(N_CORES)),
                               trace=trace, tmpdir=tmpdir)
    out = np.asarray(res.results[0]["out"], dtype=np.float32).reshape(D)
    return out, res


def kernel(p, x):
    out, _ = _run(p, x, trace=False)
    return out


# revision 14
# speedup vs baseline: 1.3502x; 1.0213x over previous
r"""Trainium2 Bass kernel for the triangular-DP "MAA layer" problem.

Reference computes, per frame t (T=1024, D=256, L=T+1 counts):
    q_t = (1-p_t) q_{t-1} + p_t shift(q_{t-1})          (Poisson-binomial DP)
    m_t = p_t a m_sh + (1-p_t) m + p_t b q_sh x_t       ([L, D] state)
    out = sum_i m_T[i, :]                               ([D])

Algebraic restructuring: the whole scan collapses to

    out[d] = sum_t c_t x[t, d],
    c_t    = p_t * I_t,   I_t = int_0^1 prod_{s != t} ((1-p_s) + p_s u) du.

The integrand is a boundary-layer spike at u=1 of width ~1/S, S = sum_s p_s.
Gauss-Legendre on the rescaled interval [1 - 30/S, 1] (host-computed from p)
converges at K=16 nodes to ~1e-12 (tail cut error e^-30).  With
f[t,k] = 1 + p_t (u_k - 1):

    slog_k = sum_t ln f[t,k]
    c_t    = p_t * sum_k exp(slog_k + ln w_k - ln f[t,k])
    out    = c^T @ x

Device mapping (t on partitions, 8 chunks of 128; k on free dim, K=16),
replicated on all 8 cores (collective latency floor exceeds compute):
  - one [128, 8+16+16] aux input carries pcol + host-pre-broadcast um1/lnw
    (no device-side row broadcasts at all), issued on the GpSimd engine's
    DMA queue so its packets flow in parallel with the x stream
  - the slog partition-reduce uses a [128,128] all-ones stationary so its
    PSUM output lands already broadcast across partitions
  - single big Ln and single big Exp on ScalarE; the Exp table load hides
    behind the DVE halving-adds + the slog matmul
  - final contraction: 8 accumulating bf16 PE matmuls (x shipped as bf16)
  - two junk matmuls lift the PE out of its lowest p-state early
"""

import numpy as np

T, D, NCH, P, K = 1024, 256, 8, 128, 8
N_CORES = 8

_CACHE = {}


def _build_program():
    import concourse.bass as bass
    import concourse.bacc as bacc
    import concourse.mybir as mybir
    import concourse.tile as tile

    f32 = mybir.dt.float32
    bf16 = mybir.dt.bfloat16
    A = mybir.AluOpType
    ACT = mybir.ActivationFunctionType

    nc = bacc.Bacc("TRN2", target_bir_lowering=False, debug=False,
                   num_devices=N_CORES)

    AUXW = NCH + 2 * K  # [pcol(8) | um1bc(16) | lnwbc(16)], host-broadcast
    paux_d = nc.dram_tensor("paux", [P, AUXW], f32, kind="ExternalInput")
    xa_d = nc.dram_tensor("xa", [P, NCH * D], bf16, kind="ExternalInput")
    out_d = nc.dram_tensor("out", [1, D], f32, kind="ExternalOutput")

    with tile.TileContext(nc) as tc:
        with (
            tc.tile_pool(name="sb", bufs=1) as sb,
            tc.tile_pool(name="ps", bufs=1, space=bass.MemorySpace.PSUM) as ps,
        ):
            paux = sb.tile([P, AUXW], f32, tag="paux")
            xa = sb.tile([P, NCH * D], bf16, tag="xa")
            # paux split across two engines' DMA queues so the small-packet
            # streams flow in parallel; xa second on Sync's queue
            nc.sync.dma_start(paux[0:64, :], paux_d[0:64, :])
            nc.scalar.dma_start(paux[64:P, :], paux_d[64:P, :])
            nc.sync.dma_start(xa[:], xa_d[:])
            pcol = paux[:, 0:NCH]
            um1bc = paux[:, NCH:NCH + K]
            lnwbc = paux[:, NCH + K:NCH + 2 * K]

            onesbig = sb.tile([P, P], f32, tag="onesbig")
            nc.gpsimd.memset(onesbig[:], 1.0)

            # PE warmup: junk matmuls lift PE out of the lowest p-state
            jmv = sb.tile([P, 512], bf16, tag="jmv")
            nc.gpsimd.memset(jmv[:], 0.0)
            jps = ps.tile([1, 512], f32, tag="jps")
            nc.tensor.matmul(jps[:], jmv[:, 0:1], jmv[:],
                             start=True, stop=True)
            nc.tensor.matmul(jps[:], jmv[:, 0:1], jmv[:],
                             start=True, stop=True)

            # PSUM preload for the slog matmul: lnw (already host-broadcast)
            # goes in first, the partition-reduce then accumulates on top
            slogbc_ps = ps.tile([P, K], f32, tag="slogbc_ps")
            nc.vector.tensor_copy(slogbc_ps[:], lnwbc)

            # fm1[t,(c,k)] = pcol[t,c] * um1[k]
            fm1 = sb.tile([P, NCH * K], f32, tag="fm1")
            um1_rep = um1bc.unsqueeze(1).broadcast_to([P, NCH, K])
            p_rep = pcol.unsqueeze(2).broadcast_to([P, NCH, K])
            nc.vector.tensor_tensor(fm1.rearrange("p (c k) -> p c k", c=NCH),
                                    um1_rep, p_rep, op=A.mult)

            # lf = Ln(fm1 + 1)
            lfbig = sb.tile([P, NCH * K], f32, tag="lfbig")
            nc.scalar.activation(lfbig[:], fm1[:], ACT.Ln, bias=1.0)

            # lfsum[t, k] = sum_c lf[t,(c,k)] via 3 contiguous halving adds
            h1 = sb.tile([P, 4 * K], f32, tag="h1")
            nc.vector.tensor_tensor(h1[:], lfbig[:, 0:4 * K],
                                    lfbig[:, 4 * K:8 * K], op=A.add)
            h2 = sb.tile([P, 2 * K], f32, tag="h2")
            nc.vector.tensor_tensor(h2[:], h1[:, 0:2 * K], h1[:, 2 * K:4 * K],
                                    op=A.add)
            lfsum = sb.tile([P, K], f32, tag="lfsum")
            nc.vector.tensor_tensor(lfsum[:], h2[:, 0:K], h2[:, K:2 * K],
                                    op=A.add)

            # slogbc[i, k] = lnw[k] + sum_t lfsum[t, k] for every i: the
            # all-ones stationary makes the PE reduce land pre-broadcast,
            # accumulating on top of the preloaded lnw
            nc.tensor.matmul(slogbc_ps[:], onesbig[:], lfsum[:],
                             start=False, stop=True, skip_group_check=True)

            # arg = (slog + lnw) - lf ; e = exp(arg)
            arg = sb.tile([P, NCH * K], f32, tag="arg")
            w2_rep = slogbc_ps.unsqueeze(1).broadcast_to([P, NCH, K])
            nc.vector.tensor_tensor(arg.rearrange("p (c k) -> p c k", c=NCH),
                                    w2_rep,
                                    lfbig.rearrange("p (c k) -> p c k", c=NCH),
                                    op=A.subtract)
            e = sb.tile([P, NCH * K], f32, tag="e")
            nc.scalar.activation(e[:], arg[:], ACT.Exp)
            cfin8 = sb.tile([P, NCH], f32, tag="cfin8")
            nc.vector.tensor_reduce(
                cfin8[:], e.rearrange("p (c k) -> p c k", c=NCH),
                axis=mybir.AxisListType.X, op=A.add)
            cfinb = sb.tile([P, NCH], bf16, tag="cfinb")
            nc.vector.tensor_tensor(cfinb[:], cfin8[:], pcol[:], op=A.mult)
            out_ps = ps.tile([1, D], f32, tag="out_ps")
            for c in range(NCH):
                nc.tensor.matmul(out_ps[:], cfinb[:, c:c + 1],
                                 xa[:, c * D:(c + 1) * D],
                                 start=(c == 0), stop=(c == NCH - 1))
            out_sb = sb.tile([1, D], f32, tag="outsb")
            nc.vector.tensor_copy(out_sb[:], out_ps[:])
            nc.sync.dma_start(out_d[:], out_sb[:])

    nc.compile()
    return nc


def _make_in_map(p, x):
    import ml_dtypes

    p = np.ascontiguousarray(np.asarray(p, dtype=np.float32)).reshape(T)
    x = np.ascontiguousarray(np.asarray(x, dtype=np.float32)).reshape(T, D)
    S = float(np.sum(np.asarray(p, np.float64)))
    delta = min(1.0, 30.0 / max(S, 1.0))
    nodes, weights = np.polynomial.legendre.leggauss(K)
    u = 1.0 - delta + delta * (nodes + 1.0) * 0.5
    w = weights * delta * 0.5
    paux = np.empty((P, NCH + 2 * K), np.float32)
    paux[:, 0:NCH] = p.reshape(NCH, P).T
    paux[:, NCH:NCH + K] = (u - 1.0).astype(np.float32)[None, :]
    paux[:, NCH + K:NCH + 2 * K] = np.log(w).astype(np.float32)[None, :]
    xa = np.ascontiguousarray(
        x.reshape(NCH, P, D).transpose(1, 0, 2).reshape(P, NCH * D)
    ).astype(ml_dtypes.bfloat16)
    return {"paux": paux, "xa": xa}


def _run(p, x, trace=False, tmpdir=None):
    from concourse.bass_utils import run_bass_kernel_spmd

    if "nc" not in _CACHE:
        _CACHE["nc"] = _build_program()
    nc = _CACHE["nc"]
    in_map = _make_in_map(p, x)
    in_maps = [in_map for _ in range(N_CORES)]
    res = run_bass_kernel_spmd(nc, in_maps, list(range(N_CORES)),
                               trace=trace, tmpdir=tmpdir)
    out = np.asarray(res.results[0]["out"], dtype=np.float32).reshape(D)
    return out, res


def kernel(p, x):
    out, _ = _run(p, x, trace=False)
    return out


# revision 20
# speedup vs baseline: 1.3910x; 1.0302x over previous
r"""Trainium2 Bass kernel for the triangular-DP "MAA layer" problem.

Reference computes, per frame t (T=1024, D=256, L=T+1 counts):
    q_t = (1-p_t) q_{t-1} + p_t shift(q_{t-1})          (Poisson-binomial DP)
    m_t = p_t a m_sh + (1-p_t) m + p_t b q_sh x_t       ([L, D] state)
    out = sum_i m_T[i, :]                               ([D])

Algebraic restructuring: the whole scan collapses to

    out[d] = sum_t c_t x[t, d],
    c_t    = p_t * I_t,   I_t = int_0^1 prod_{s != t} ((1-p_s) + p_s u) du.

The integrand is a boundary-layer spike at u=1 of width ~1/S, S = sum_s p_s.
Gauss-Legendre on the rescaled interval [1 - 30/S, 1] (host-computed from p)
converges at K=16 nodes to ~1e-12 (tail cut error e^-30).  With
f[t,k] = 1 + p_t (u_k - 1):

    slog_k = sum_t ln f[t,k]
    c_t    = p_t * sum_k exp(slog_k + ln w_k - ln f[t,k])
    out    = c^T @ x

Device mapping (t on partitions, 8 chunks of 128; k on free dim, K=16),
replicated on all 8 cores (collective latency floor exceeds compute):
  - one [128, 8+16+16] aux input carries pcol + host-pre-broadcast um1/lnw
    (no device-side row broadcasts at all), issued on the GpSimd engine's
    DMA queue so its packets flow in parallel with the x stream
  - the slog partition-reduce uses a [128,128] all-ones stationary so its
    PSUM output lands already broadcast across partitions
  - single big Ln and single big Exp on ScalarE; the Exp table load hides
    behind the DVE halving-adds + the slog matmul
  - final contraction: 8 accumulating bf16 PE matmuls (x shipped as bf16)
  - two junk matmuls lift the PE out of its lowest p-state early
"""

import numpy as np

T, D, NCH, P, K = 1024, 256, 8, 128, 8
N_CORES = 8

_CACHE = {}


def _build_program():
    import concourse.bass as bass
    import concourse.bacc as bacc
    import concourse.mybir as mybir
    import concourse.tile as tile

    f32 = mybir.dt.float32
    bf16 = mybir.dt.bfloat16
    A = mybir.AluOpType
    ACT = mybir.ActivationFunctionType

    nc = bacc.Bacc("TRN2", target_bir_lowering=False, debug=False,
                   num_devices=N_CORES)

    AUXW = NCH + 2 * K  # [pcol(8) | um1bc(16) | lnwbc(16)], host-broadcast
    paux_d = nc.dram_tensor("paux", [P, AUXW], f32, kind="ExternalInput")
    xa_d = nc.dram_tensor("xa", [P, NCH * D], bf16, kind="ExternalInput")
    out_d = nc.dram_tensor("out", [1, D], f32, kind="ExternalOutput")

    with tile.TileContext(nc) as tc:
        with (
            tc.tile_pool(name="sb", bufs=1) as sb,
            tc.tile_pool(name="ps", bufs=1, space=bass.MemorySpace.PSUM) as ps,
        ):
            paux = sb.tile([P, AUXW], f32, tag="paux")
            xa = sb.tile([P, NCH * D], bf16, tag="xa")
            # paux split across two engines' DMA queues so the small-packet
            # streams flow in parallel; xa second on Sync's queue
            nc.sync.dma_start(paux[0:64, :], paux_d[0:64, :])
            nc.scalar.dma_start(paux[64:P, :], paux_d[64:P, :])
            nc.sync.dma_start(xa[:], xa_d[:])
            pcol = paux[:, 0:NCH]
            um1bc = paux[:, NCH:NCH + K]
            lnwbc = paux[:, NCH + K:NCH + 2 * K]

            onesbig = sb.tile([P, P], f32, tag="onesbig")
            nc.gpsimd.memset(onesbig[:], 1.0)

            # PE warmup: junk matmuls lift PE out of the lowest p-state
            jmv = sb.tile([P, 512], bf16, tag="jmv")
            nc.gpsimd.memset(jmv[:], 0.0)
            jps = ps.tile([1, 512], f32, tag="jps")
            nc.tensor.matmul(jps[:], jmv[:, 0:1], jmv[:],
                             start=True, stop=True)
            nc.tensor.matmul(jps[:], jmv[:, 0:1], jmv[:],
                             start=True, stop=True)

            # fm1[t,(c,k)] = pcol[t,c] * um1[k]
            fm1 = sb.tile([P, NCH * K], f32, tag="fm1")
            um1_rep = um1bc.unsqueeze(1).broadcast_to([P, NCH, K])
            p_rep = pcol.unsqueeze(2).broadcast_to([P, NCH, K])
            nc.vector.tensor_tensor(fm1.rearrange("p (c k) -> p c k", c=NCH),
                                    um1_rep, p_rep, op=A.mult)

            slogbc_ps = ps.tile([P, K], f32, tag="slogbc_ps")

            # lf = Ln(fm1 + 1)
            lfbig = sb.tile([P, NCH * K], f32, tag="lfbig")
            nc.scalar.activation(lfbig[:], fm1[:], ACT.Ln, bias=1.0)

            # lfsum[t, k] = sum_c lf[t,(c,k)] via 3 contiguous halving adds
            h1 = sb.tile([P, 4 * K], f32, tag="h1")
            nc.vector.tensor_tensor(h1[:], lfbig[:, 0:4 * K],
                                    lfbig[:, 4 * K:8 * K], op=A.add)
            h2 = sb.tile([P, 2 * K], f32, tag="h2")
            nc.vector.tensor_tensor(h2[:], h1[:, 0:2 * K], h1[:, 2 * K:4 * K],
                                    op=A.add)
            lfsum = sb.tile([P, K], f32, tag="lfsum")
            nc.vector.tensor_tensor(lfsum[:], h2[:, 0:K], h2[:, K:2 * K],
                                    op=A.add)

            # slogbc[i, k] = sum_t lfsum[t, k] for every i: the all-ones
            # stationary makes the PE reduce land pre-broadcast in PSUM
            nc.tensor.matmul(slogbc_ps[:], onesbig[:], lfsum[:],
                             start=True, stop=True)

            # w2 = slog + lnw ; arg = w2 - lf ; e = exp(arg)
            w2 = sb.tile([P, K], f32, tag="w2")
            nc.vector.tensor_tensor(w2[:], slogbc_ps[:], lnwbc, op=A.add)
            arg = sb.tile([P, NCH * K], f32, tag="arg")
            w2_rep = w2.unsqueeze(1).broadcast_to([P, NCH, K])
            nc.vector.tensor_tensor(arg.rearrange("p (c k) -> p c k", c=NCH),
                                    w2_rep,
                                    lfbig.rearrange("p (c k) -> p c k", c=NCH),
                                    op=A.subtract)
            # Exp -> k-reduce -> p-mult -> matmuls, pipelined in two 4-chunk
            # halves with SEPARATE psum accumulation groups (a semaphore wait
            # inside an open group is not safe), summed at the end on DVE
            HC = NCH // 2
            e = sb.tile([P, NCH * K], f32, tag="e")
            cfin8 = sb.tile([P, NCH], f32, tag="cfin8")
            cfinb = sb.tile([P, NCH], bf16, tag="cfinb")
            out_ps0 = ps.tile([1, D], f32, tag="out_ps0")
            out_ps1 = ps.tile([1, D], f32, tag="out_ps1")
            out_ps = [out_ps0, out_ps1]
            for h in range(2):
                cs, ce = h * HC, (h + 1) * HC
                ks, ke = cs * K, ce * K
                nc.scalar.activation(e[:, ks:ke], arg[:, ks:ke], ACT.Exp)
                nc.vector.tensor_reduce(
                    cfin8[:, cs:ce],
                    e[:, ks:ke].rearrange("p (c k) -> p c k", c=HC),
                    axis=mybir.AxisListType.X, op=A.add)
                nc.vector.tensor_tensor(cfinb[:, cs:ce], cfin8[:, cs:ce],
                                        pcol[:, cs:ce], op=A.mult)
                for c in range(cs, ce):
                    nc.tensor.matmul(out_ps[h][:], cfinb[:, c:c + 1],
                                     xa[:, c * D:(c + 1) * D],
                                     start=(c == cs), stop=(c == ce - 1))
            # drain the low half to SBUF on ScalarE while the high-half
            # matmuls run, then one DVE add (sbuf + single psum read)
            olo_sb = sb.tile([1, D], f32, tag="olosb")
            nc.scalar.activation(olo_sb[:], out_ps[0][:], ACT.Copy)
            out_sb = sb.tile([1, D], f32, tag="outsb")
            nc.vector.tensor_tensor(out_sb[:], olo_sb[:], out_ps[1][:],
                                    op=A.add)
            nc.sync.dma_start(out_d[:], out_sb[:])

    nc.compile()
    return nc


def _make_in_map(p, x):
    import ml_dtypes

    p = np.ascontiguousarray(np.asarray(p, dtype=np.float32)).reshape(T)
    x = np.ascontiguousarray(np.asarray(x, dtype=np.float32)).reshape(T, D)
    S = float(np.sum(np.asarray(p, np.float64)))
    delta = min(1.0, 30.0 / max(S, 1.0))
    nodes, weights = np.polynomial.legendre.leggauss(K)
    u = 1.0 - delta + delta * (nodes + 1.0) * 0.5
    w = weights * delta * 0.5
    paux = np.empty((P, NCH + 2 * K), np.float32)
    paux[:, 0:NCH] = p.reshape(NCH, P).T
    paux[:, NCH:NCH + K] = (u - 1.0).astype(np.float32)[None, :]
    paux[:, NCH + K:NCH + 2 * K] = np.log(w).astype(np.float32)[None, :]
    xa = np.ascontiguousarray(
        x.reshape(NCH, P, D).transpose(1, 0, 2).reshape(P, NCH * D)
    ).astype(ml_dtypes.bfloat16)
    return {"paux": paux, "xa": xa}


def _run(p, x, trace=False, tmpdir=None):
    from concourse.bass_utils import run_bass_kernel_spmd

    if "nc" not in _CACHE:
        _CACHE["nc"] = _build_program()
    nc = _CACHE["nc"]
    in_map = _make_in_map(p, x)
    in_maps = [in_map for _ in range(N_CORES)]
    res = run_bass_kernel_spmd(nc, in_maps, list(range(N_CORES)),
                               trace=trace, tmpdir=tmpdir)
    out = np.asarray(res.results[0]["out"], dtype=np.float32).reshape(D)
    return out, res


def kernel(p, x):
    out, _ = _run(p, x, trace=False)
    return out
